# revision 1
# baseline (speedup 1.0000x reference)
"""Trainium2 Bass kernel for qk-layernorm attention (dense transformer block).

Sharding: 8 cores = 2 batches x 4 head-groups (4 heads each).  Each core
computes qkv projection (its heads only), qk-layernorm, attention, and a
partial output projection for its head slice; the host sums the 4 partials
per batch and adds b_proj.

Per-core layout strategy (all matmuls fp32r, full-speed 1 cycle/row):
 - everything kept transposed: q^T,k^T [d, n] tiles with 2 heads packed per
   128-partition tile; v in natural [n, d] layout
 - LN stats (mean / E[x^2]) via all-ones-matrix matmuls -> broadcast rows in
   PSUM for free; gamma folded into W_qkv columns on the host
 - S^T = k_ln^T.T @ q_hat per head with row-packed (tile_position) pairs
 - softmax without max subtraction (|logits| <= 8 since q,k are unit-var)
 - denominator via ones-column matmul accumulated alongside attn@v
 - attn@v col-packed pairs -> out^T in PSUM, normalized late by 1/denom
"""

import numpy as np

DIM = 1024
HEADS = 16
HD = 64
B = 2
N = 2048
EPS = 1e-6
N_CORES = 8
HEADS_PER_CORE = 4
PAIRS = 2          # head pairs per core
CC = 8             # contraction chunks of 128 over DIM
NT = N // 128      # 16 n/m tiles
NCH = N // 512     # 4 chunks of 512
SCALE = HD ** -0.5

_prog_cache = {}


def _build_program():
    import concourse.bass as bass
    import concourse.tile as tile
    from concourse import mybir, bacc

    F32 = mybir.dt.float32
    F32R = mybir.dt.float32r
    Act = mybir.ActivationFunctionType
    Alu = mybir.AluOpType

    nc = bacc.Bacc("TRN2", target_bir_lowering=False, debug=False,
                   num_devices=N_CORES)

    # ---- DRAM I/O ----
    xT_d = nc.dram_tensor("xT", [DIM, N], F32R, kind="ExternalInput").ap()
    wqk_d = nc.dram_tensor("wqk", [DIM, 512], F32R, kind="ExternalInput").ap()
    wv_d = nc.dram_tensor("wv", [DIM, 256], F32R, kind="ExternalInput").ap()
    wp_d = nc.dram_tensor("wp", [256, DIM], F32R, kind="ExternalInput").ap()
    smu_q_d = nc.dram_tensor("smu_q", [128, 128], F32R, kind="ExternalInput").ap()
    ssq_q_d = nc.dram_tensor("ssq_q", [128, 128], F32R, kind="ExternalInput").ap()
    smu_k_d = nc.dram_tensor("smu_k", [128, 128], F32R, kind="ExternalInput").ap()
    ssq_k_d = nc.dram_tensor("ssq_k", [128, 128], F32R, kind="ExternalInput").ap()
    bsel_d = nc.dram_tensor("bsel", [128, 64], F32R, kind="ExternalInput").ap()
    zer_d = nc.dram_tensor("zer", [128, 1024], F32R, kind="ExternalInput").ap()
    gq_d = nc.dram_tensor("gq", [128, 1], F32, kind="ExternalInput").ap()
    gk_d = nc.dram_tensor("gk", [128, 1], F32, kind="ExternalInput").ap()
    ones_d = nc.dram_tensor("ones", [128, 64], F32R, kind="ExternalInput").ap()
    ident_d = nc.dram_tensor("ident", [128, 128], F32R, kind="ExternalInput").ap()
    y_d = nc.dram_tensor("y", [N, DIM], F32, kind="ExternalOutput").ap()

    with tile.TileContext(nc) as tc:
        with tc.tile_pool(name="wts", bufs=1) as wts, \
             tc.tile_pool(name="persist", bufs=1) as persist:
            # ---- persistent SBUF tensors ----
            wqk = wts.tile([128, CC * 512], F32R)       # [c-chunk, 4 o-tiles x 128]
            nc.sync.dma_start(wqk[:].rearrange("p (cc o) -> p cc o", cc=CC),
                              wqk_d.rearrange("(cc p) o -> p cc o", p=128))
            wv = wts.tile([128, CC * 256], F32R)
            nc.sync.dma_start(wv[:].rearrange("p (cc o) -> p cc o", cc=CC),
                              wv_d.rearrange("(cc p) o -> p cc o", p=128))
            wp = wts.tile([128, 2 * DIM], F32R)
            nc.sync.dma_start(wp[:].rearrange("p (pc o) -> p pc o", pc=2),
                              wp_d.rearrange("(pc p) o -> p pc o", p=128))
            smu = [wts.tile([128, 128], F32R, tag=f"smu{t}", name=f"smu{t}") for t in range(2)]
            ssq = [wts.tile([128, 128], F32R, tag=f"ssq{t}", name=f"ssq{t}") for t in range(2)]
            nc.sync.dma_start(smu[0][:], smu_q_d[:])
            nc.sync.dma_start(ssq[0][:], ssq_q_d[:])
            nc.sync.dma_start(smu[1][:], smu_k_d[:])
            nc.sync.dma_start(ssq[1][:], ssq_k_d[:])
            gcol = [wts.tile([128, 1], F32, tag=f"g{t}", name=f"g{t}") for t in range(2)]
            nc.sync.dma_start(gcol[0][:], gq_d[:])
            nc.sync.dma_start(gcol[1][:], gk_d[:])
            ones = wts.tile([128, 64], F32R)
            nc.sync.dma_start(ones[:], ones_d[:])
            bsel = wts.tile([128, 64], F32R)
            nc.sync.dma_start(bsel[:], bsel_d[:])
            denA = wts.tile([128, 1024], F32R)
            nc.sync.dma_start(denA[:], zer_d[:])
            denB = wts.tile([128, 1024], F32R)
            nc.sync.dma_start(denB[:], zer_d[:])
            epsb = wts.tile([128, 1], F32)
            nc.gpsimd.memset(epsb[:], EPS)

            # qk[0],qk[1]: q pair tiles; qk[2],qk[3]: k pair tiles  [d-pair, n]
            qk = [persist.tile([128, N], F32R, tag=f"qk{i}", name=f"qk{i}") for i in range(4)]
            vT = [persist.tile([128, N], F32R, tag=f"vT{p}", name=f"vT{p}") for p in range(PAIRS)]
            v_sb = persist.tile([128, NT * 260], F32R)   # [m, nt*(4 heads x [64 v | 1])]
            onorm = [persist.tile([128, N], F32R, tag=f"on{p}", name=f"on{p}") for p in range(PAIRS)]
            ident = wts.tile([128, 128], F32R)
            nc.sync.dma_start(ident[:], ident_d[:])
            v_ones = v_sb[:].rearrange("p (nt b c) -> p nt b c", b=4, c=65)[:, :, :, 64:65]
            nc.sync.dma_start(
                v_ones, ones_d.rearrange("p (nt b) -> p nt b", b=4)[:, :, :, None])

            # ================= Phase 1: qkv projection =================
            # 3 passes over xT chunks (PSUM limits to 2 o-tiles per pass)
            xT_r = xT_d.rearrange("(cc p) n -> p cc n", p=128)

            def proj_all(xp, ps, jobs, tmp):
                # dest = W_cols.T @ x^T; single pass over xT, n-chunk outer,
                # one psum accumulator per job (6 jobs -> 6 banks); qk-layernorm
                # stats + apply run inline per chunk in the 2 spare banks so
                # the serial DVE/ACT chains overlap the next chunk's matmuls
                for nch in range(NCH):
                    sl = slice(nch * 512, (nch + 1) * 512)
                    accs = [ps.tile([128, 512], F32, tag=f"acc{j}",
                                    name=f"acc{j}") for j in range(len(jobs))]
                    xt = xp.tile([128, CC * 512], F32R, tag="xt", name="xt")
                    nc.sync.dma_start(
                        xt[:].rearrange("p (cc n) -> p cc n", cc=CC),
                        xT_r[:, :, nch * 512:(nch + 1) * 512])
                    for cc in range(CC):
                        for j, (dest, w, wstride, woff) in enumerate(jobs):
                            nc.tensor.matmul(
                                accs[j][:],
                                w[:, cc * wstride + woff:cc * wstride + woff + 128],
                                xt[:, cc * 512:(cc + 1) * 512],
                                start=(cc == 0), stop=(cc == CC - 1))
                    for j, (dest, w, wstride, woff) in enumerate(jobs):
                        nc.vector.tensor_copy(dest[:, sl], accs[j][:])
                    for p in range(PAIRS):
                        for t in range(2):      # 0 = q, 1 = k
                            src = qk[2 * t + p]
                            sqc = tmp.tile([128, 512], F32R, tag="sqc",
                                           name="sqc")
                            nc.vector.tensor_mul(sqc[:], src[:, sl], src[:, sl])
                            pmu = ps.tile([128, 512], F32, tag="mu", name="pmu")
                            psq = ps.tile([128, 512], F32, tag="sqp", name="psq")
                            nc.tensor.matmul(pmu[:], smu[t][:], src[:, sl],
                                             start=True, stop=True)
                            nc.tensor.matmul(psq[:], ssq[t][:], sqc[:],
                                             start=True, stop=True)
                            rs = tmp.tile([128, 512], F32, tag="rs", name="rs")
                            nc.scalar.activation(rs[:], pmu[:], Act.Square)
                            nc.vector.tensor_sub(rs[:], psq[:], rs[:])
                            nc.scalar.activation(rs[:], rs[:], Act.Sqrt,
                                                 bias=epsb[:])
                            nc.vector.reciprocal(rs[:], rs[:])
                            t1 = tmp.tile([128, 512], F32, tag="t1", name="t1")
                            nc.vector.scalar_tensor_tensor(
                                t1[:], pmu[:], gcol[t][:], src[:, sl],
                                op0=Alu.mult, op1=Alu.subtract)
                            nc.vector.scalar_tensor_tensor(
                                src[:, sl], t1[:], -1.0, rs[:],
                                op0=Alu.mult, op1=Alu.mult)

            with tc.tile_pool(name="xT", bufs=2) as xp, \
                 tc.tile_pool(name="p2tmp", bufs=2) as tmp:
                with tc.tile_pool(name="ps_p1", bufs=1, space="PSUM") as ps:
                    proj_all(xp, ps, [(qk[0], wqk, 512, 0),
                                      (qk[2], wqk, 512, 256),
                                      (qk[1], wqk, 512, 128),
                                      (qk[3], wqk, 512, 384),
                                      (vT[0], wv, 256, 0),
                                      (vT[1], wv, 256, 128)], tmp)
            # ================= Phase 3: attention =================
            def proj_tiles(p4, ps4, nts):
                for nt in nts:
                    py = ps4.tile([128, 1024], F32, tag="s", bufs=1, name="py")
                    for oc in range(2):
                        for p in range(PAIRS):
                            nc.tensor.matmul(
                                py[:, oc * 512:(oc + 1) * 512],
                                onorm[p][:, nt * 128:(nt + 1) * 128],
                                wp[:, p * 1024 + oc * 512:p * 1024 + (oc + 1) * 512],
                                start=(p == 0), stop=(p == PAIRS - 1))
                    yt = p4.tile([128, 1024], F32, tag="yt", bufs=3, name="yt")
                    nc.scalar.copy(yt[:], py[:])
                    nc.sync.dma_start(y_d[nt * 128:(nt + 1) * 128, :], yt[:])

            with tc.tile_pool(name="p3", bufs=2) as p3, \
                 tc.tile_pool(name="ps3", bufs=1, space="PSUM") as ps3:
                for mt in range(NT):
                    for p in range(PAIRS):
                        ptr = ps3.tile([128, 128], F32R, tag="s", bufs=1,
                                       name="ptr")
                        nc.tensor.transpose(
                            ptr[:], vT[p][:, mt * 128:(mt + 1) * 128], ident[:])
                        base = mt * 260 + p * 130
                        nc.vector.tensor_copy(
                            v_sb[:, base:base + 64], ptr[:, 0:64])
                        nc.vector.tensor_copy(
                            v_sb[:, base + 65:base + 129], ptr[:, 64:128])
                for nh in range(2):            # halves of n (1024 each)
                    for p in range(PAIRS):
                        qt, kt = qk[p], qk[2 + p]
                        poh = [ps3.tile([128, 1024], F32, tag=f"po{h}",
                                        name=f"po{h}") for h in range(2)]
                        for mt in range(NT):
                            first, last = (mt == 0), (mt == NT - 1)
                            psS = ps3.tile([128, 2048], F32, tag="s",
                                           bufs=1, name="psS")
                            for h in range(2):     # head halves (rows 0/64)
                                hs = slice(h * 64, (h + 1) * 64)
                                for nq in range(2):
                                    nsl = slice(nh * 1024 + nq * 512,
                                                nh * 1024 + (nq + 1) * 512)
                                    nc.tensor.matmul(
                                        psS[:, h * 1024 + nq * 512:
                                            h * 1024 + (nq + 1) * 512],
                                        kt[hs, mt * 128:(mt + 1) * 128],
                                        qt[hs, nsl], start=True, stop=True)
                            eS = p3.tile([128, 2048], F32R, tag="eS",
                                         bufs=3, name="eS")
                            nc.scalar.activation(eS[:], psS[:], Act.Exp,
                                                 scale=float(SCALE))
                            for h in range(2):
                                vsl = v_sb[:, mt * 260 + (p * 2 + h) * 65:
                                           mt * 260 + (p * 2 + h) * 65 + 65]
                                for nq in range(2):
                                    nc.tensor.matmul(
                                        poh[h][0:65, nq * 512:(nq + 1) * 512],
                                        vsl,
                                        eS[:, h * 1024 + nq * 512:
                                           h * 1024 + (nq + 1) * 512],
                                        start=first, stop=last)
                        # denominators -> broadcast -> reciprocal -> normalize
                        nc.vector.tensor_copy(denA[64:65, :], poh[0][64:65, :])
                        nc.vector.tensor_copy(denB[64:65, :], poh[1][64:65, :])
                        rds = []
                        for h, dent in ((0, denA), (1, denB)):
                            pb = ps3.tile([128, 1024], F32, tag="s", bufs=1,
                                          name=f"pb{h}")
                            for nq in range(2):
                                nc.tensor.matmul(
                                    pb[0:64, nq * 512:(nq + 1) * 512], bsel[:],
                                    dent[:, nq * 512:(nq + 1) * 512],
                                    start=True, stop=True)
                            rd = p3.tile([128, 1024], F32, tag=f"rd{h}",
                                         name=f"rd{h}")
                            nc.vector.reciprocal(rd[0:64, :], pb[0:64, :])
                            rds.append(rd)
                        rdA, rdB = rds
                        nc.vector.tensor_mul(
                            onorm[p][0:64, nh * 1024:(nh + 1) * 1024],
                            poh[0][0:64, :], rdA[0:64, :])
                        tmpB = p3.tile([128, 1024], F32R, tag="tmpB")
                        nc.vector.tensor_mul(
                            tmpB[0:64, :], poh[1][0:64, :], rdB[0:64, :])
                        nc.sync.dma_start(
                            onorm[p][64:128, nh * 1024:(nh + 1) * 1024],
                            tmpB[0:64, :])
                    with tc.tile_pool(name=f"p4_{nh}", bufs=1) as p4:
                        proj_tiles(p4, ps3, range(nh * 8, (nh + 1) * 8))

    nc.compile()
    return nc


def _prep_core_inputs(x, W_qkv, q_gamma, k_gamma, W_proj):
    """Host-side sharding + layout prep. Returns list of 8 in_maps."""
    f32 = np.float32
    blkdiag = np.kron(np.eye(2, dtype=f32), np.ones((64, 64), f32))
    bsel = np.zeros((128, 64), f32)
    bsel[64, :] = 1.0
    in_maps = []
    for core in range(N_CORES):
        b, g = core // 4, core % 4
        heads = [4 * g + j for j in range(HEADS_PER_CORE)]
        qcols = np.concatenate(
            [(W_qkv[h * HD:(h + 1) * HD, :] * q_gamma[:, None]).T for h in heads],
            axis=1)
        kcols = np.concatenate(
            [(W_qkv[DIM + h * HD:DIM + (h + 1) * HD, :] * k_gamma[:, None]).T
             for h in heads], axis=1)
        wqk = np.ascontiguousarray(
            np.concatenate([qcols, kcols], axis=1), dtype=f32)
        wv = np.ascontiguousarray(
            np.concatenate(
                [W_qkv[2 * DIM + h * HD:2 * DIM + (h + 1) * HD, :].T
                 for h in heads], axis=1), dtype=f32)
        wp = np.ascontiguousarray(
            W_proj[:, heads[0] * HD:(heads[-1] + 1) * HD].T, dtype=f32)
        g2q = np.tile(q_gamma, 2).astype(f32)
        g2k = np.tile(k_gamma, 2).astype(f32)
        in_maps.append({
            "xT": np.ascontiguousarray(x[b].T, dtype=f32),
            "wqk": wqk, "wv": wv, "wp": wp,
            "smu_q": (blkdiag * (1.0 / (64.0 * g2q))[:, None]).astype(f32),
            "ssq_q": (blkdiag * (1.0 / (64.0 * g2q * g2q))[:, None]).astype(f32),
            "smu_k": (blkdiag * (1.0 / (64.0 * g2k))[:, None]).astype(f32),
            "ssq_k": (blkdiag * (1.0 / (64.0 * g2k * g2k))[:, None]).astype(f32),
            "bsel": bsel,
            "zer": np.zeros((128, 1024), f32),
            "gq": g2q[:, None].copy(), "gk": g2k[:, None].copy(),
            "ones": np.ones((128, 64), f32),
            "ident": np.eye(128, dtype=f32),
        })
    return in_maps


def _numpy_fallback(x, W_qkv, q_gamma, q_beta, k_gamma, k_beta, W_proj, b_proj):
    def ln(t, gamma, beta):
        mu = t.mean(-1, keepdims=True)
        var = ((t - mu) ** 2).mean(-1, keepdims=True)
        return (t - mu) / np.sqrt(var + EPS) * gamma + beta
    Bs, Ns, C = x.shape
    qkv = np.einsum('bnc,oc->bno', x, W_qkv)
    qkv = qkv.reshape(Bs, Ns, 3, HEADS, HD).transpose(2, 0, 3, 1, 4)
    q, k, v = ln(qkv[0], q_gamma, q_beta), ln(qkv[1], k_gamma, k_beta), qkv[2]
    s = np.einsum('bhnd,bhmd->bhnm', q * SCALE, k)
    s = np.exp(s - s.max(-1, keepdims=True))
    p = s / s.sum(-1, keepdims=True)
    o = np.einsum('bhnm,bhmd->bhnd', p, v)
    o = o.transpose(0, 2, 1, 3).reshape(Bs, Ns, C)
    return (np.einsum('bnc,oc->bno', o, W_proj) + b_proj).astype(np.float32)


def kernel(x, W_qkv, q_gamma, q_beta, k_gamma, k_beta, W_proj, b_proj):
    x = np.asarray(x, np.float32)
    W_qkv = np.asarray(W_qkv, np.float32)
    q_gamma = np.asarray(q_gamma, np.float32)
    q_beta = np.asarray(q_beta, np.float32)
    k_gamma = np.asarray(k_gamma, np.float32)
    k_beta = np.asarray(k_beta, np.float32)
    W_proj = np.asarray(W_proj, np.float32)
    b_proj = np.asarray(b_proj, np.float32)

    if np.any(q_beta != 0) or np.any(k_beta != 0):
        # beta terms are not wired into the device kernel (reference always
        # uses beta = 0); fall back to exact host computation
        return _numpy_fallback(x, W_qkv, q_gamma, q_beta, k_gamma, k_beta,
                               W_proj, b_proj)

    from concourse import bass_utils

    if "prog" not in _prog_cache:
        _prog_cache["prog"] = _build_program()
    nc = _prog_cache["prog"]

    in_maps = _prep_core_inputs(x, W_qkv, q_gamma, k_gamma, W_proj)
    res = bass_utils.run_bass_kernel_spmd(nc, in_maps, list(range(N_CORES)))

    out = np.empty((B, N, DIM), np.float32)
    for b in range(B):
        acc = res.results[4 * b + 0]["y"].astype(np.float32).copy()
        for g in range(1, 4):
            acc += res.results[4 * b + g]["y"]
        out[b] = acc + b_proj
    return out



# revision 10
# speedup vs baseline: 1.3267x; 1.3267x over previous
"""Trainium2 Bass kernel for qk-layernorm attention (dense transformer block).

Sharding: 8 cores = 2 batches x 4 head-groups (4 heads each).  Each core
computes qkv projection (its heads only), qk-layernorm, attention, and a
partial output projection for its head slice; the host sums the 4 partials
per batch and adds b_proj.

v2 (bf16): all matmuls run in bf16 (fp32 PSUM accumulation) - fp32 matmuls
on TRN2 run at half rate (fp32_mode=HIGH) and kept the PE cold.  Changes vs
the fp32r baseline:
 - all matmul operands bf16 (weights cast on host, activations cast on chip)
 - v projected directly in [n, d] layout (x chunk stationary) - no PE
   transposes
 - attention PV pairs two heads via PE column tiling (64+64 stationary);
   softmax denominators via 1-col stationary matmuls into 32-aligned PSUM
   partition strips of one bank
 - LN rsqrt via ACT ln/exp (one table set for the whole kernel); softmax
   1/den via DVE reciprocal_approx_fast after a matmul partition-broadcast
 - output projection PSUM tiles copied out on DVE and DMA'd per (nt, oc)
"""

import numpy as np
import ml_dtypes

DIM = 1024
HEADS = 16
HD = 64
B = 2
N = 2048
EPS = 1e-6
N_CORES = 8
HEADS_PER_CORE = 4
PAIRS = 2          # head pairs per core
CC = 8             # contraction chunks of 128 over DIM
NT = N // 128      # 16 n/m tiles
NCH = N // 512     # 4 chunks of 512
SCALE = HD ** -0.5

BF16 = ml_dtypes.bfloat16

_prog_cache = {}


def _build_program():
    import concourse.bass as bass
    import concourse.tile as tile
    from concourse import mybir, bacc

    F32 = mybir.dt.float32
    BF = mybir.dt.bfloat16
    Act = mybir.ActivationFunctionType
    Alu = mybir.AluOpType

    nc = bacc.Bacc("TRN2", target_bir_lowering=False, debug=False,
                   num_devices=N_CORES)

    # ---- DRAM I/O ----
    xT_d = nc.dram_tensor("xT", [DIM, N], BF, kind="ExternalInput").ap()
    wqk_d = nc.dram_tensor("wqk", [DIM, 512], BF, kind="ExternalInput").ap()
    wv_d = nc.dram_tensor("wv", [DIM, 256], BF, kind="ExternalInput").ap()
    wp_d = nc.dram_tensor("wp", [256, DIM], BF, kind="ExternalInput").ap()
    smu_q_d = nc.dram_tensor("smu_q", [128, 128], BF, kind="ExternalInput").ap()
    ssq_q_d = nc.dram_tensor("ssq_q", [128, 128], BF, kind="ExternalInput").ap()
    smu_k_d = nc.dram_tensor("smu_k", [128, 128], BF, kind="ExternalInput").ap()
    ssq_k_d = nc.dram_tensor("ssq_k", [128, 128], BF, kind="ExternalInput").ap()
    sel_d = nc.dram_tensor("sel", [128, 256], BF, kind="ExternalInput").ap()
    gq_d = nc.dram_tensor("gq", [128, 1], F32, kind="ExternalInput").ap()
    gk_d = nc.dram_tensor("gk", [128, 1], F32, kind="ExternalInput").ap()
    ones_d = nc.dram_tensor("ones", [128, 1], BF, kind="ExternalInput").ap()
    dones_d = nc.dram_tensor("dones", [128, 512], BF, kind="ExternalInput").ap()
    y_d = nc.dram_tensor("y", [N, DIM], F32, kind="ExternalOutput").ap()

    with tile.TileContext(nc) as tc:
        with tc.tile_pool(name="wts", bufs=1) as wts, \
             tc.tile_pool(name="persist", bufs=1) as persist:
            # ---- persistent SBUF tensors ----
            wqk = wts.tile([128, CC * 512], BF)         # [c, cc x (q256|k256)]
            wv = wts.tile([128, CC * 256], BF)          # [c, cc x 256]
            wp = wts.tile([128, 2 * DIM], BF)           # [d-pair, pc x 1024]
            nc.sync.dma_start(wqk[:].rearrange("p (cc o) -> p cc o", cc=CC),
                              wqk_d.rearrange("(cc p) o -> p cc o", p=128))
            nc.sync.dma_start(wv[:].rearrange("p (cc o) -> p cc o", cc=CC),
                              wv_d.rearrange("(cc p) o -> p cc o", p=128))
            nc.sync.dma_start(wp[:].rearrange("p (pc o) -> p pc o", pc=2),
                              wp_d.rearrange("(pc p) o -> p pc o", p=128))
            smu = [wts.tile([128, 128], BF, tag=f"smu{t}", name=f"smu{t}")
                   for t in range(2)]
            ssq = [wts.tile([128, 128], BF, tag=f"ssq{t}", name=f"ssq{t}")
                   for t in range(2)]
            nc.sync.dma_start(smu[0][:], smu_q_d[:])
            nc.sync.dma_start(ssq[0][:], ssq_q_d[:])
            nc.sync.dma_start(smu[1][:], smu_k_d[:])
            nc.sync.dma_start(ssq[1][:], ssq_k_d[:])
            gcol = [wts.tile([128, 1], F32, tag=f"g{t}", name=f"g{t}")
                    for t in range(2)]
            nc.sync.dma_start(gcol[0][:], gq_d[:])
            nc.sync.dma_start(gcol[1][:], gk_d[:])
            sel = wts.tile([128, 256], BF)              # [k, nq x 128]
            nc.sync.dma_start(sel[:], sel_d[:])
            ones1 = wts.tile([128, 1], BF)
            nc.sync.dma_start(ones1[:], ones_d[:])
            epsb = wts.tile([128, 1], F32)
            nc.gpsimd.memset(epsb[:], EPS)

            # full xT resident in SBUF: [c-part, cc, n]
            xt = persist.tile([128, CC * N], BF)
            xt_v = xt[:].rearrange("p (cc n) -> p cc n", cc=CC)
            xT_r = xT_d.rearrange("(cc p) n -> p cc n", p=128)

            # qk[0],qk[1]: q pair tiles; qk[2],qk[3]: k pair tiles [d-pair, n]
            qk = [persist.tile([128, N], BF, tag=f"qk{i}", name=f"qk{i}")
                  for i in range(4)]
            v_sb = persist.tile([128, NT * 256], BF)    # [m, mt x (4 heads x 64)]
            onorm = [persist.tile([128, N], BF, tag=f"on{p}", name=f"on{p}")
                     for p in range(PAIRS)]
            den_sb = wts.tile([128, 512], BF)
            nc.sync.dma_start(den_sb[:], dones_d[:])

            # ============ Phase 1: qk projection + qk-layernorm ============
            with tc.tile_pool(name="p1s", bufs=2) as p1s, \
                 tc.tile_pool(name="ps1", bufs=1, space="PSUM") as ps1:
                for nch in range(NCH):
                    sl = slice(nch * 512, (nch + 1) * 512)
                    nc.sync.dma_start(xt_v[:, :, sl], xT_r[:, :, sl])
                    # two sub-blocks: (q pair_p, k pair_p) for p = 0, 1
                    for p in range(PAIRS):
                        accq = ps1.tile([128, 512], F32, tag="accq", bufs=2,
                                        name="accq")
                        acck = ps1.tile([128, 512], F32, tag="acck", bufs=2,
                                        name="acck")
                        for cc in range(CC):
                            nc.tensor.matmul(
                                accq[:], wqk[:, cc * 512 + p * 128:
                                             cc * 512 + p * 128 + 128],
                                xt[:, cc * N + nch * 512:cc * N + nch * 512 + 512],
                                start=(cc == 0), stop=(cc == CC - 1))
                        for cc in range(CC):
                            nc.tensor.matmul(
                                acck[:], wqk[:, cc * 512 + 256 + p * 128:
                                             cc * 512 + 256 + p * 128 + 128],
                                xt[:, cc * N + nch * 512:cc * N + nch * 512 + 512],
                                start=(cc == 0), stop=(cc == CC - 1))
                        # qk-layernorm for both tiles of this pair
                        for t, acc in ((0, accq), (1, acck)):
                            dest = qk[2 * t + p]
                            qb = p1s.tile([128, 512], BF, tag="qb", name="qb")
                            nc.vector.tensor_copy(qb[:], acc[:])
                            sqc = p1s.tile([128, 512], BF, tag="sqc", name="sqc")
                            nc.scalar.activation(sqc[:], acc[:], Act.Square)
                            pmu = ps1.tile([128, 512], F32, tag="pmu", bufs=2,
                                           name="pmu")
                            psq = ps1.tile([128, 512], F32, tag="psq", bufs=2,
                                           name="psq")
                            nc.tensor.matmul(pmu[:], smu[t][:], qb[:],
                                             start=True, stop=True)
                            nc.tensor.matmul(psq[:], ssq[t][:], sqc[:],
                                             start=True, stop=True)
                            mu2 = p1s.tile([128, 512], F32, tag="mu2", name="mu2")
                            nc.scalar.activation(mu2[:], pmu[:], Act.Square)
                            var = p1s.tile([128, 512], F32, tag="var", name="var")
                            nc.vector.tensor_sub(var[:], psq[:], mu2[:])
                            # rs = 1/sqrt(var+eps) = exp(-0.5 * ln(var+eps))
                            lnv = p1s.tile([128, 512], F32, tag="lnv", name="lnv")
                            nc.scalar.activation(lnv[:], var[:], Act.Ln,
                                                 bias=epsb[:])
                            rs = p1s.tile([128, 512], F32, tag="rs", name="rs")
                            nc.scalar.activation(rs[:], lnv[:], Act.Exp,
                                                 scale=-0.5)
                            t1 = p1s.tile([128, 512], F32, tag="t1", name="t1")
                            nc.vector.scalar_tensor_tensor(
                                t1[:], pmu[:], gcol[t][:], qb[:],
                                op0=Alu.mult, op1=Alu.subtract)
                            nc.vector.scalar_tensor_tensor(
                                dest[:, sl], t1[:], -1.0, rs[:],
                                op0=Alu.mult, op1=Alu.mult)

            # ============ Phase 1.5: v projection (x chunk stationary) ======
            with tc.tile_pool(name="p15", bufs=2) as p15, \
                 tc.tile_pool(name="ps15", bufs=1, space="PSUM") as ps15:
                for mt in range(NT):
                    vacc = ps15.tile([128, 256], F32, tag="vacc", bufs=2,
                                     name="vacc")
                    for cc in range(CC):
                        nc.tensor.matmul(
                            vacc[:], xt[:, cc * N + mt * 128:cc * N + mt * 128 + 128],
                            wv[:, cc * 256:(cc + 1) * 256],
                            start=(cc == 0), stop=(cc == CC - 1))
                    nc.vector.tensor_copy(v_sb[:, mt * 256:(mt + 1) * 256],
                                          vacc[:])

            # ================= Phase 3: attention =================
            with tc.tile_pool(name="p3", bufs=2) as p3, \
                 tc.tile_pool(name="ps3", bufs=1, space="PSUM") as ps3:
                for nh in range(2):            # halves of n (1024 each)
                    for p in range(PAIRS):
                        qt, kt = qk[p], qk[2 + p]
                        # o^T accumulator: partitions 0-63 h0, 64-127 h1.
                        # Col-tiled groups share banks, so accumulation uses
                        # explicit memset + start=False (one start per bank
                        # would zero sibling groups' partials).
                        poh = ps3.tile([128, 1024], F32, tag="poh", bufs=1,
                                       name="poh")
                        nc.vector.memset(poh[:], 0.0)
                        # denominators: partition strip 32*(2h+nq), cols=n
                        den = ps3.tile([128, 512], F32, tag="pbpy", bufs=2,
                                       name="den")
                        nc.vector.memset(den[:], 0.0)
                        for mt in range(NT):
                            first, last = (mt == 0), (mt == NT - 1)
                            psS = ps3.tile([128, 2048], F32, tag="psS",
                                           bufs=1, name="psS")
                            for nq in range(2):
                                for h in range(2):   # row-group pairs
                                    hs = slice(h * 64, (h + 1) * 64)
                                    nc.tensor.matmul(
                                        psS[:, h * 1024 + nq * 512:
                                            h * 1024 + nq * 512 + 512],
                                        kt[hs, mt * 128:(mt + 1) * 128],
                                        qt[hs, nh * 1024 + nq * 512:
                                           nh * 1024 + nq * 512 + 512],
                                        start=True, stop=True)
                            eS = p3.tile([128, 2048], BF, tag="eS",
                                         bufs=3, name="eS")
                            nc.scalar.activation(eS[:], psS[:], Act.Exp,
                                                 scale=float(SCALE))
                            for nq in range(2):
                                for h in range(2):   # col-group pairs
                                    nc.tensor.matmul(
                                        poh[h * 64:(h + 1) * 64,
                                            nq * 512:(nq + 1) * 512],
                                        v_sb[:, mt * 256 + (2 * p + h) * 64:
                                             mt * 256 + (2 * p + h) * 64 + 64],
                                        eS[:, h * 1024 + nq * 512:
                                           h * 1024 + nq * 512 + 512],
                                        start=False, stop=False,
                                        skip_group_check=True)
                                for h in range(2):   # denominator strips
                                    s = (2 * h + nq) * 32
                                    nc.tensor.matmul(
                                        den[s:s + 1, :], ones1[:, 0:1],
                                        eS[:, h * 1024 + nq * 512:
                                           h * 1024 + nq * 512 + 512],
                                        start=False, stop=False,
                                        skip_group_check=True,
                                        tile_position=(0, s))
                        # 1/den -> broadcast -> normalize
                        for h in range(2):
                            for nq in range(2):
                                s = (2 * h + nq) * 32
                                nc.vector.tensor_copy(den_sb[s:s + 1, :],
                                                      den[s:s + 1, :])
                        for nq in range(2):
                            pb = ps3.tile([128, 512], F32, tag="pbpy", bufs=2,
                                          name="pb")
                            nc.tensor.matmul(pb[:],
                                             sel[:, nq * 128:(nq + 1) * 128],
                                             den_sb[:], start=True, stop=True)
                            rd = p3.tile([128, 512], F32, tag="rd", name="rd")
                            nc.vector.reciprocal_approx_fast(rd[:], pb[:])
                            nc.vector.tensor_mul(
                                onorm[p][:, nh * 1024 + nq * 512:
                                       nh * 1024 + nq * 512 + 512],
                                poh[:, nq * 512:(nq + 1) * 512], rd[:])
                    # -------- output projection for this n-half --------
                    for nt in range(nh * 8, (nh + 1) * 8):
                        for oc in range(2):
                            py = ps3.tile([128, 512], F32, tag="pbpy", bufs=2,
                                          name="py")
                            for p in range(PAIRS):
                                nc.tensor.matmul(
                                    py[:],
                                    onorm[p][:, nt * 128:(nt + 1) * 128],
                                    wp[:, p * 1024 + oc * 512:
                                       p * 1024 + oc * 512 + 512],
                                    start=(p == 0), stop=(p == PAIRS - 1))
                            yt = p3.tile([128, 512], F32, tag="yt", bufs=3,
                                         name="yt")
                            nc.vector.tensor_copy(yt[:], py[:])
                            nc.sync.dma_start(
                                y_d[nt * 128:(nt + 1) * 128,
                                    oc * 512:(oc + 1) * 512], yt[:])

    nc.compile()
    return nc


def _prep_core_inputs(x, W_qkv, q_gamma, k_gamma, W_proj):
    """Host-side sharding + layout prep. Returns list of 8 in_maps."""
    f32 = np.float32
    blkdiag = np.kron(np.eye(2, dtype=f32), np.ones((64, 64), f32))
    # sel: broadcast den strips to partition halves, per nq
    sel = np.zeros((128, 256), f32)
    sel[0, 0:64] = 1.0       # nq0: strip 0  (h0) -> partitions 0-63
    sel[64, 64:128] = 1.0    # nq0: strip 64 (h1) -> partitions 64-127
    sel[32, 128:192] = 1.0   # nq1: strip 32 (h0)
    sel[96, 192:256] = 1.0   # nq1: strip 96 (h1)
    in_maps = []
    for core in range(N_CORES):
        b, g = core // 4, core % 4
        heads = [4 * g + j for j in range(HEADS_PER_CORE)]
        qcols = np.concatenate(
            [(W_qkv[h * HD:(h + 1) * HD, :] * q_gamma[:, None]).T for h in heads],
            axis=1)
        kcols = np.concatenate(
            [(W_qkv[DIM + h * HD:DIM + (h + 1) * HD, :] * k_gamma[:, None]).T
             for h in heads], axis=1)
        wqk = np.ascontiguousarray(
            np.concatenate([qcols, kcols], axis=1), dtype=f32)
        wv = np.ascontiguousarray(
            np.concatenate(
                [W_qkv[2 * DIM + h * HD:2 * DIM + (h + 1) * HD, :].T
                 for h in heads], axis=1), dtype=f32)
        wp = np.ascontiguousarray(
            W_proj[:, heads[0] * HD:(heads[-1] + 1) * HD].T, dtype=f32)
        g2q = np.tile(q_gamma, 2).astype(f32)
        g2k = np.tile(k_gamma, 2).astype(f32)
        in_maps.append({
            "xT": np.ascontiguousarray(x[b].T).astype(BF16),
            "wqk": wqk.astype(BF16), "wv": wv.astype(BF16),
            "wp": wp.astype(BF16),
            "smu_q": (blkdiag * (1.0 / (64.0 * g2q))[:, None]).astype(BF16),
            "ssq_q": (blkdiag * (1.0 / (64.0 * g2q * g2q))[:, None]).astype(BF16),
            "smu_k": (blkdiag * (1.0 / (64.0 * g2k))[:, None]).astype(BF16),
            "ssq_k": (blkdiag * (1.0 / (64.0 * g2k * g2k))[:, None]).astype(BF16),
            "sel": sel.astype(BF16),
            "gq": g2q[:, None].copy(), "gk": g2k[:, None].copy(),
            "ones": np.ones((128, 1), f32).astype(BF16),
            "dones": np.ones((128, 512), f32).astype(BF16),
        })
    return in_maps


def _numpy_fallback(x, W_qkv, q_gamma, q_beta, k_gamma, k_beta, W_proj, b_proj):
    def ln(t, gamma, beta):
        mu = t.mean(-1, keepdims=True)
        var = ((t - mu) ** 2).mean(-1, keepdims=True)
        return (t - mu) / np.sqrt(var + EPS) * gamma + beta
    Bs, Ns, C = x.shape
    qkv = np.einsum('bnc,oc->bno', x, W_qkv)
    qkv = qkv.reshape(Bs, Ns, 3, HEADS, HD).transpose(2, 0, 3, 1, 4)
    q, k, v = ln(qkv[0], q_gamma, q_beta), ln(qkv[1], k_gamma, k_beta), qkv[2]
    s = np.einsum('bhnd,bhmd->bhnm', q * SCALE, k)
    s = np.exp(s - s.max(-1, keepdims=True))
    p = s / s.sum(-1, keepdims=True)
    o = np.einsum('bhnm,bhmd->bhnd', p, v)
    o = o.transpose(0, 2, 1, 3).reshape(Bs, Ns, C)
    return (np.einsum('bnc,oc->bno', o, W_proj) + b_proj).astype(np.float32)


def kernel(x, W_qkv, q_gamma, q_beta, k_gamma, k_beta, W_proj, b_proj):
    x = np.asarray(x, np.float32)
    W_qkv = np.asarray(W_qkv, np.float32)
    q_gamma = np.asarray(q_gamma, np.float32)
    q_beta = np.asarray(q_beta, np.float32)
    k_gamma = np.asarray(k_gamma, np.float32)
    k_beta = np.asarray(k_beta, np.float32)
    W_proj = np.asarray(W_proj, np.float32)
    b_proj = np.asarray(b_proj, np.float32)

    if np.any(q_beta != 0) or np.any(k_beta != 0):
        # beta terms are not wired into the device kernel (reference always
        # uses beta = 0); fall back to exact host computation
        return _numpy_fallback(x, W_qkv, q_gamma, q_beta, k_gamma, k_beta,
                               W_proj, b_proj)

    from concourse import bass_utils

    if "prog" not in _prog_cache:
        _prog_cache["prog"] = _build_program()
    nc = _prog_cache["prog"]

    in_maps = _prep_core_inputs(x, W_qkv, q_gamma, k_gamma, W_proj)
    res = bass_utils.run_bass_kernel_spmd(nc, in_maps, list(range(N_CORES)))

    out = np.empty((B, N, DIM), np.float32)
    for b in range(B):
        acc = res.results[4 * b + 0]["y"].astype(np.float32).copy()
        for g in range(1, 4):
            acc += res.results[4 * b + g]["y"]
        out[b] = acc + b_proj
    return out
